# revision 7
# baseline (speedup 1.0000x reference)
"""Trainium2 Bass kernel for nn_CustomLoss_69999376990919.

Math: the reference's A-inner-product modified Gram-Schmidt + projection
collapses to per-sample 4x4 Gram matrices
    G[s] = P_s diag(a_s) P_s^T,   R[s] = P_s diag(a_s) T_s
after which   loss = mean_s (4 - h^2 tr(R^T G^{-1} R)) / 4
(Cholesky of G == Gram-Schmidt in exact arithmetic; <v,Av> > 0 always holds
since coefficients > 0).  The device streams all inputs (memory-bound) and
produces G/R; the tiny 4x4 solves run on the host in float64.

Sharding: pure data parallelism, batch axis 0 split across 8 cores
(64 samples each), processed as 2 groups of 32 (PSUM block = 4*32 = 128).

Layout strategy (the knobs are DMA packet size, SBUF-AXI write bytes, and
matmul operand contiguity; the per-core floor is the HBM read stream
37.75 MB @ ~358 GB/s ~= 105 us):
  - The host pre-permutes each core's slab (pure fp32 layout change; the
    device still reads every input byte from HBM) to
        pt     [p=128, g=2, f=128, 256]   (n = p*128 + f) where the 256
               columns are preds (j,s) then targs (s,m) interleaved per f
        coeff  [p=128, g=2, f=128, s=32]
    so every DMA reads multi-KB contiguous runs per partition (4 KB packets
    at SDMA line rate instead of 512 B runs at ~40 ns/packet) and every
    matmul operand slice is a flat contiguous 2-D AP.
  - All loads are SWDGE cast-DMAs fp32->bf16 (>=1 KB writes, no sub-512 B
    read-modify-write), halving the SBUF-AXI write-side bytes: ~87 us.
  - DVE computes W = a (.) P into a flat [128, f, (j s)] tile with
    contiguous reads and writes (2x bf16 perf mode eligible).
  - TensorE per f: one LDW(W[f]) (flat 128-col bf16 -> FWL) + ONE matmul
    with the 256-column moving slice pt[f] accumulating G and R blocks
    side by side in PSUM: 256 matmuls total, burst ~2-3 us per 16-f chunk
    against a ~6.8 us/chunk DMA cadence, so the PE never idles past the
    HAM MID window and stays at K=8/8.
  Everything hides under the ~105-111 us HBM stream.
bf16 is safe: the loss is 1 - O(1e-4); quantization moves it by ~1e-9 rel.
"""

import numpy as np

import concourse.bacc as bacc
from contextlib import ExitStack

import concourse.tile as tile
from concourse import mybir
from concourse.bass_utils import run_bass_kernel_spmd

B, C, N = 512, 4, 16384
H = 0.0078125  # grid spacing; A = diag(h^2 * coefficients)
NCORES = 8
SPC = B // NCORES  # 64 samples per core
NG = 2             # groups per core
GS = SPC // NG     # 32 samples per group
P = 128            # SBUF partitions; n = p*128 + f
F = N // P         # 128 f-steps
X = 2 * C * GS     # 256 moving columns: preds (j,s) ++ targs (s,m)
XA = X + GS        # + 32 coeff columns appended per f
# chunk taper: short chunks at the end keep the final PE burst tiny
CHUNKS = [16] * 6 + [8] * 4

_CACHE = {}


def _build_bass():
    nc = bacc.Bacc(trn_type="TRN2")
    ptin = nc.dram_tensor("ptin", [P, NG * F * XA], mybir.dt.float32,
                          kind="ExternalInput")
    out = nc.dram_tensor("gr_out", [P, NG * X], mybir.dt.float32,
                         kind="ExternalOutput")

    pt_v = ptin[:].rearrange("p (g f x) -> p g f x", g=NG, f=F)

    with tile.TileContext(nc) as tc, ExitStack() as ctx:
        pt_pool = ctx.enter_context(tc.tile_pool(name="pt_pool", bufs=8))
        w_pool = ctx.enter_context(tc.tile_pool(name="w_pool", bufs=2))
        outs = ctx.enter_context(tc.tile_pool(name="outs", bufs=1))
        psums = ctx.enter_context(tc.tile_pool(name="psums", bufs=2, space="PSUM"))

        out_stage = outs.tile([P, NG * X], mybir.dt.float32)

        for g in range(NG):
            w16 = w_pool.tile([P, F, C * GS], mybir.dt.bfloat16, tag="w16",
                              name=f"w16_{g}")
            psum = psums.tile([P, X], mybir.dt.float32, tag="ps",
                              name=f"ps_{g}")

            f0 = 0
            for fc, fcl in enumerate(CHUNKS):
                fsl = slice(f0, f0 + fcl)
                pt16 = pt_pool.tile([P, fcl, XA], mybir.dt.bfloat16, tag="pt16",
                                    name=f"pt16_{g}_{fc}")
                nc.gpsimd.dma_start(out=pt16[:], in_=pt_v[:, g, fsl, :])

                # W = a * p, all APs contiguous per j (a broadcast by loop)
                for j in range(C):
                    nc.vector.tensor_mul(
                        w16[:, fsl, j * GS : (j + 1) * GS],
                        pt16[:, :, X:XA],
                        pt16[:, :, j * GS : (j + 1) * GS],
                    )

                for fl in range(fcl):
                    f = f0 + fl
                    nc.tensor.matmul(
                        psum[:],
                        w16[:, f, :],        # stationary [128, 128] contiguous
                        pt16[:, fl, 0:X],    # moving [128, 256] contiguous
                        start=(f == 0),
                        stop=(f == F - 1),
                    )
                f0 += fcl

            nc.scalar.copy(out=out_stage[:, g * X : (g + 1) * X], in_=psum[:])
            nc.sync.dma_start(
                out=out[:, g * X : (g + 1) * X],
                in_=out_stage[:, g * X : (g + 1) * X],
            )

    if not nc.is_finalized():
        nc.finalize()
    return nc


def _get_nc():
    if "nc" not in _CACHE:
        _CACHE["nc"] = _build_bass()
    return _CACHE["nc"]


def kernel(coefficients, predictions, targets):
    co = np.asarray(coefficients, dtype=np.float32)
    pr = np.asarray(predictions, dtype=np.float32)
    tg = np.asarray(targets, dtype=np.float32)
    assert co.shape == (B, N) and pr.shape == (B, C, N) and tg.shape == (B, N, C)

    # Host-side pure permutation into DMA/matmul-friendly layouts (fp32;
    # the device still streams every byte).  c=core, g=group, s=sample in
    # group, p=partition, f (n = p*128 + f), j=class, m=target column.
    co_p = co.reshape(NCORES, NG, GS, P, F).transpose(0, 3, 1, 4, 2)
    pr_p = pr.reshape(NCORES, NG, GS, C, P, F).transpose(0, 4, 1, 5, 3, 2)
    tg_p = tg.reshape(NCORES, NG, GS, P, F, C).transpose(0, 3, 1, 4, 2, 5)
    pt = np.concatenate(
        [pr_p.reshape(NCORES, P, NG, F, C * GS),
         tg_p.reshape(NCORES, P, NG, F, GS * C),
         co_p.reshape(NCORES, P, NG, F, GS)],
        axis=-1,
    )  # [c, p, g, f, 288]
    pt = np.ascontiguousarray(pt)

    nc = _get_nc()
    in_maps = []
    for c in range(NCORES):
        in_maps.append({
            "ptin": pt[c].reshape(P, NG * F * XA),
        })

    res = run_bass_kernel_spmd(nc, in_maps, core_ids=list(range(NCORES)))
    _CACHE["last"] = res

    # host epilogue: extract per-sample 4x4 G/R diagonals, fp64 solve
    G = np.empty((B, C, C), np.float64)
    R = np.empty((B, C, C), np.float64)
    for c in range(NCORES):
        o = np.asarray(res.results[c]["gr_out"], dtype=np.float64)
        for g in range(NG):
            bg = o[:, g * X : g * X + C * GS].reshape(C, GS, C, GS)
            br = o[:, g * X + C * GS : (g + 1) * X].reshape(C, GS, GS, C)
            s0 = c * SPC + g * GS
            G[s0 : s0 + GS] = np.einsum("isjs->sij", bg)
            R[s0 : s0 + GS] = np.einsum("issm->sim", br)

    G = 0.5 * (G + np.swapaxes(G, 1, 2))
    Xs = np.linalg.solve(G, R)
    val = (H * H) * np.einsum("bim,bim->b", R, Xs)
    loss = np.mean((4.0 - val) / 4.0)
    return np.float32(loss)


# revision 8
# speedup vs baseline: 1.1514x; 1.1514x over previous
"""Trainium2 Bass kernel for nn_CustomLoss_69999376990919.

Math: the reference's A-inner-product modified Gram-Schmidt + projection
collapses to per-sample 4x4 Gram matrices
    G[s] = P_s diag(a_s) P_s^T,   R[s] = P_s diag(a_s) T_s
after which   loss = mean_s (4 - h^2 tr(R^T G^{-1} R)) / 4
(Cholesky of G == Gram-Schmidt in exact arithmetic; <v,Av> > 0 always holds
since coefficients > 0).  The device streams all inputs (memory-bound) and
produces G/R; the tiny 4x4 solves run on the host in float64.

Sharding: pure data parallelism, batch axis 0 split across 8 cores
(64 samples each), processed as 2 groups of 32 (PSUM block = 4*32 = 128).

Layout strategy (the knobs are DMA packet size, SBUF-AXI write bytes, and
matmul operand contiguity; the per-core floor is the input stream:
37.75 MB of HBM reads at the 16-SDMA-engine read-beat ceiling ~= 87 us):
  - The host pre-permutes each core's slab (pure fp32 layout change; the
    device still reads every input byte from HBM) to
        pt     [p=128, g=2, f=128, 256]   (n = p*128 + f) where the 256
               columns are preds (j,s) then targs (s,m) interleaved per f
        coeff  [p=128, g=2, f=128, s=32]
    so every DMA reads multi-KB contiguous runs per partition (4 KB packets
    at SDMA line rate instead of 512 B runs at ~40 ns/packet) and every
    matmul operand slice is a flat contiguous 2-D AP.
  - All loads are SWDGE cast-DMAs fp32->bf16 (>=1 KB writes, no sub-512 B
    read-modify-write), halving the SBUF-AXI write-side bytes (that port
    budget is 2:1-muxed with the sibling NeuronCore, ~218 GB/s).
  - DVE computes W = a (.) P into a flat [128, f, (j s)] tile with
    contiguous reads and writes.
  - TensorE per f: one LDW(W[f]) (flat 128-col bf16) + ONE matmul with the
    256-column moving slice pt[f] accumulating the G and R blocks side by
    side in PSUM: 256 matmuls total.
  - Deep tile pools (bufs=10 per 16-f chunk) decouple the DMA stream from
    the chunk-paced consumers so the SDMA engines never starve.
  Everything hides under the ~92 us input stream; measured ~109-115 us
  end-to-end including the ~9 us engine preamble and the drain tail.
bf16 is safe: the loss is 1 - O(1e-4); quantization moves it by ~1e-9 rel.
"""

import numpy as np

import concourse.bacc as bacc
from contextlib import ExitStack

import concourse.tile as tile
from concourse import mybir
from concourse.bass_utils import run_bass_kernel_spmd

B, C, N = 512, 4, 16384
H = 0.0078125  # grid spacing; A = diag(h^2 * coefficients)
NCORES = 8
SPC = B // NCORES  # 64 samples per core
NG = 2             # groups per core
GS = SPC // NG     # 32 samples per group
P = 128            # SBUF partitions; n = p*128 + f
F = N // P         # 128 f-steps
FC = 8             # f-chunks per group
FCL = F // FC      # 16 f-steps per chunk
X = 2 * C * GS     # 256 moving columns: preds (j,s) ++ targs (s,m)

_CACHE = {}


def _build_bass():
    nc = bacc.Bacc(trn_type="TRN2")
    coeff = nc.dram_tensor("coeff", [P, NG * F * GS], mybir.dt.float32,
                           kind="ExternalInput")
    ptin = nc.dram_tensor("ptin", [P, NG * F * X], mybir.dt.float32,
                          kind="ExternalInput")
    out = nc.dram_tensor("gr_out", [P, NG * X], mybir.dt.float32,
                         kind="ExternalOutput")

    coeff_v = coeff[:].rearrange("p (g f s) -> p g f s", g=NG, f=F)
    pt_v = ptin[:].rearrange("p (g f x) -> p g f x", g=NG, f=F)

    with tile.TileContext(nc) as tc, ExitStack() as ctx:
        a_pool = ctx.enter_context(tc.tile_pool(name="a_pool", bufs=10))
        pt_pool = ctx.enter_context(tc.tile_pool(name="pt_pool", bufs=10))
        w_pool = ctx.enter_context(tc.tile_pool(name="w_pool", bufs=2))
        outs = ctx.enter_context(tc.tile_pool(name="outs", bufs=1))
        psums = ctx.enter_context(tc.tile_pool(name="psums", bufs=2, space="PSUM"))

        out_stage = outs.tile([P, NG * X], mybir.dt.float32)

        for g in range(NG):
            w16 = w_pool.tile([P, F, C * GS], mybir.dt.bfloat16, tag="w16",
                              name=f"w16_{g}")
            psum = psums.tile([P, X], mybir.dt.float32, tag="ps",
                              name=f"ps_{g}")

            for fc in range(FC):
                fsl = slice(fc * FCL, (fc + 1) * FCL)
                a16 = a_pool.tile([P, FCL, GS], mybir.dt.bfloat16, tag="a16",
                                  name=f"a16_{g}_{fc}")
                pt16 = pt_pool.tile([P, FCL, X], mybir.dt.bfloat16, tag="pt16",
                                    name=f"pt16_{g}_{fc}")
                nc.gpsimd.dma_start(out=a16[:], in_=coeff_v[:, g, fsl, :])
                nc.gpsimd.dma_start(out=pt16[:], in_=pt_v[:, g, fsl, :])

                # W = a * p, all APs contiguous per j (a broadcast by loop)
                for j in range(C):
                    nc.vector.tensor_mul(
                        w16[:, fsl, j * GS : (j + 1) * GS],
                        a16[:],
                        pt16[:, :, j * GS : (j + 1) * GS],
                    )

                for fl in range(FCL):
                    f = fc * FCL + fl
                    nc.tensor.matmul(
                        psum[:],
                        w16[:, f, :],     # stationary [128, 128] contiguous
                        pt16[:, fl, :],   # moving [128, 256] contiguous
                        start=(f == 0),
                        stop=(f == F - 1),
                    )

            nc.scalar.copy(out=out_stage[:, g * X : (g + 1) * X], in_=psum[:])
            nc.sync.dma_start(
                out=out[:, g * X : (g + 1) * X],
                in_=out_stage[:, g * X : (g + 1) * X],
            )

    if not nc.is_finalized():
        nc.finalize()
    return nc


def _get_nc():
    if "nc" not in _CACHE:
        _CACHE["nc"] = _build_bass()
    return _CACHE["nc"]


def kernel(coefficients, predictions, targets):
    co = np.asarray(coefficients, dtype=np.float32)
    pr = np.asarray(predictions, dtype=np.float32)
    tg = np.asarray(targets, dtype=np.float32)
    assert co.shape == (B, N) and pr.shape == (B, C, N) and tg.shape == (B, N, C)

    # Host-side pure permutation into DMA/matmul-friendly layouts (fp32;
    # the device still streams every byte).  c=core, g=group, s=sample in
    # group, p=partition, f (n = p*128 + f), j=class, m=target column.
    co_p = np.ascontiguousarray(
        co.reshape(NCORES, NG, GS, P, F).transpose(0, 3, 1, 4, 2)
    )  # [c, p, g, f, s]
    pr_p = pr.reshape(NCORES, NG, GS, C, P, F).transpose(0, 4, 1, 5, 3, 2)
    tg_p = tg.reshape(NCORES, NG, GS, P, F, C).transpose(0, 3, 1, 4, 2, 5)
    pt = np.concatenate(
        [pr_p.reshape(NCORES, P, NG, F, C * GS),
         tg_p.reshape(NCORES, P, NG, F, GS * C)],
        axis=-1,
    )  # [c, p, g, f, 256]
    pt = np.ascontiguousarray(pt)

    nc = _get_nc()
    in_maps = []
    for c in range(NCORES):
        in_maps.append({
            "coeff": co_p[c].reshape(P, NG * F * GS),
            "ptin": pt[c].reshape(P, NG * F * X),
        })

    res = run_bass_kernel_spmd(nc, in_maps, core_ids=list(range(NCORES)))
    _CACHE["last"] = res

    # host epilogue: extract per-sample 4x4 G/R diagonals, fp64 solve
    G = np.empty((B, C, C), np.float64)
    R = np.empty((B, C, C), np.float64)
    for c in range(NCORES):
        o = np.asarray(res.results[c]["gr_out"], dtype=np.float64)
        for g in range(NG):
            bg = o[:, g * X : g * X + C * GS].reshape(C, GS, C, GS)
            br = o[:, g * X + C * GS : (g + 1) * X].reshape(C, GS, GS, C)
            s0 = c * SPC + g * GS
            G[s0 : s0 + GS] = np.einsum("isjs->sij", bg)
            R[s0 : s0 + GS] = np.einsum("issm->sim", br)

    G = 0.5 * (G + np.swapaxes(G, 1, 2))
    Xs = np.linalg.solve(G, R)
    val = (H * H) * np.einsum("bim,bim->b", R, Xs)
    loss = np.mean((4.0 - val) / 4.0)
    return np.float32(loss)
